# revision 1
# baseline (speedup 1.0000x reference)
"""Self-contained Trainium2 Bass kernel for sliding-window attention.

Problem (hardcoded): B=1, S=8192, dim=1024, H=16 heads, D=64 head dim,
window=512, fp32 I/O.  y = (softmax(mask(rope(xWq^T) rope(xWk^T)^T / 8)) xWv^T) Wo^T

Strategy: sequence-parallel over 8 NeuronCores. Each core owns 1024 query
rows and additionally recomputes K/V for the 512-row halo to its left
(core 0's halo is zero-padded and neutralized via a per-core "vones"
column so no collective is needed).  All matmuls run in bf16 (fp32 PSUM
accumulation); sliding-window causality is applied by extra rank-128
mask matmuls accumulated into the score PSUM before the exp.

Layouts (per core):
  xT    [1024(d), 1536(s)]  x^T shard incl. halo (bf16)
  wq/wk [1024(d), 1024(e')] Wq^T / Wk^T with a per-head even/odd column
                            permutation (rope pair de-interleave: head h's
                            rows are [evens(32) | odds(32)])
  wv    [1024(d), 1024(e)]  Wv^T (no permutation), wo = Wo^T
  Q^T/K^T are produced in [e', s] layout (weight-stationary matmuls) so
  attention needs no transposes: scores are computed transposed,
  S^T[k, q], the softmax denominator comes free from a ones-column
  appended to V, and PV directly yields o^T[e, q] — the lhsT of the
  output projection.  Rope runs on the vector engine using 32-periodic
  cos/sin tables (equal-base operand reads + partition-shifted writes).
"""
import sys

sys.path.insert(0, "/opt/trn_rl_repo")

import numpy as np
import ml_dtypes

import concourse.bass as bass
import concourse.mybir as mybir
from concourse import bacc
from concourse.tile import TileContext
from concourse.bass_utils import run_bass_kernel_spmd

BF = ml_dtypes.bfloat16
NCORES = 8
S, DIM, H, D, W = 8192, 1024, 16, 64, 512
SL = S // NCORES          # 1024 own rows / core
SK = SL + W               # 1536 rows incl. left halo
P = 128
NKT = SK // P             # 12 kv tiles
NQB = SL // P             # 8 query tiles
dt = mybir.dt

_compiled = {}


def _build(dbg=False):
    nc = bacc.Bacc("TRN2", target_bir_lowering=False, debug=False,
                   num_devices=NCORES)
    def param(name, shape, dtype=dt.bfloat16, out=False):
        return nc.declare_dram_parameter(name, shape, dtype, isOutput=out)

    xt = param("xt", [DIM, SK])
    wq = param("wq", [DIM, DIM])
    wk = param("wk", [DIM, DIM])
    wv = param("wv", [DIM, DIM])
    wo = param("wo", [DIM, DIM])
    ropc = param("ropc", [P, SK])
    rops = param("rops", [P, SK])
    vone = param("vone", [SK, 1])
    uold = param("uold", [P, P])
    udia = param("udia", [P, P])
    negi = param("negi", [P, P])
    out = param("out", [SL, DIM], dt.float32, out=True)
    dbg_outs = None
    if dbg:
        dbg_outs = {
            "d_qt": param("d_qt", [P, 8 * SL], out=True),
            "d_kt": param("d_kt", [P, 8 * SK], out=True),
            "d_v": param("d_v", [P, NKT * H * 80], out=True),
            "d_ot": param("d_ot", [P, 8 * SL], out=True),
                                }

    with TileContext(nc) as tc:
        _body(nc, tc, xt, wq, wk, wv, wo, ropc, rops, vone, uold, udia,
              negi, out, dbg_outs)
    nc.compile()
    return nc


def _brd2(ap_slice, n):
    """Insert a stride-0 middle free dim of size n into a [p, c] AP."""
    return bass.AP(tensor=ap_slice.tensor, offset=ap_slice.offset,
                   ap=[ap_slice.ap[0], [0, n], ap_slice.ap[1]])


VA = 80   # V_aug columns: 64 V + 1 ones + pad (32B-aligned stride)


def _body(nc, tc, xt, wq, wk, wv, wo, ropc, rops, vone, uold, udia, negi,
          out, dbg_outs=None):
    f32, bf16 = dt.float32, dt.bfloat16
    mult, add = mybir.AluOpType.mult, mybir.AluOpType.add

    with tc.tile_pool(name="persist", bufs=1) as per:
        # long-lived SBUF tensors
        v_sb = per.tile([P, NKT, H, VA], bf16)    # V_aug: [V(64)|ones|pad]
        qt_sb = per.tile([P, 8, SL], bf16)        # Q^T (rope'd, sigma layout)
        kt_sb = per.tile([P, 8, SK], bf16)        # K^T
        ot_sb = per.tile([P, 8, SL], bf16)        # o^T (normalized)
        ropc_sb = per.tile([P, SK], bf16)
        rops_sb = per.tile([P, SK], bf16)
        uold_sb = per.tile([P, P], bf16)
        udia_sb = per.tile([P, P], bf16)
        negi_sb = per.tile([P, P], bf16)
        vone_sb = per.tile([P, NKT], bf16)

        with tc.tile_pool(name="xtp", bufs=1) as xtp, \
             tc.tile_pool(name="proj", bufs=1, space="PSUM") as projp, \
             tc.tile_pool(name="raw", bufs=2) as rawp, \
             tc.tile_pool(name="mro", bufs=1) as mrp, \
             tc.tile_pool(name="wqk", bufs=1) as wqkp, \
             tc.tile_pool(name="pt", bufs=3) as ptp, \
             tc.tile_pool(name="st", bufs=2, space="PSUM") as stp, \
             tc.tile_pool(name="pv", bufs=3, space="PSUM") as pvp, \
             tc.tile_pool(name="osb", bufs=2) as osbp, \
             tc.tile_pool(name="eps", bufs=1) as epsp:
            xt_sb = xtp.tile([P, 8, SK], bf16)
            xt_r = xt.ap().rearrange("(d p) s -> p d s", p=P)
            _eng = [nc.sync, nc.scalar, nc.gpsimd]
            for d in range(8):
                _eng[d % 3].dma_start(out=xt_sb[:, d, :], in_=xt_r[:, d, :])

            wk_sb = wqkp.tile([P, 8, DIM], bf16)
            wq_sb = wqkp.tile([P, 8, DIM], bf16)
            wo_sb = wqkp.tile([P, 8, DIM], bf16)
            wk_r = wk.ap().rearrange("(d p) e -> p d e", p=P)
            wq_r = wq.ap().rearrange("(d p) e -> p d e", p=P)
            for d in range(8):
                _eng[(d + 1) % 3].dma_start(out=wk_sb[:, d, :],
                                            in_=wk_r[:, d, :])
            for d in range(8):
                _eng[(d + 2) % 3].dma_start(out=wq_sb[:, d, :],
                                            in_=wq_r[:, d, :])

            nc.sync.dma_start(out=ropc_sb, in_=ropc[:, :])
            nc.sync.dma_start(out=rops_sb, in_=rops[:, :])
            nc.sync.dma_start(out=uold_sb, in_=uold[:, :])
            nc.sync.dma_start(out=udia_sb, in_=udia[:, :])
            nc.sync.dma_start(out=negi_sb, in_=negi[:, :])
            nc.sync.dma_start(out=vone_sb,
                          in_=vone.ap().rearrange("(k p) o -> p (k o)", p=P))
            # ones column of V_aug (per-core halo-validity mask)
            for kt_i in range(NKT):
                nc.vector.tensor_copy(
                    out=v_sb[:, kt_i, :, 64],
                    in_=_brd2(vone_sb[:, kt_i:kt_i + 1], H))

            nc.sync.dma_start(
                out=wo_sb, in_=wo.ap().rearrange("(e p) n -> p e n", p=P))

            def rope(raw, dst, nsc, c0):
                # raw [P, 1, nsc] bf16 -> dst (rope'd).  Tables are
                # 32-row-periodic, so shifted reads use the same base for
                # both operands; writes are partition-shifted.
                mcos = mrp.tile([P, nsc], bf16, name="mcos", tag="mcos")
                msw = mrp.tile([P, nsc], bf16, name="msw", tag="msw")
                cseg = slice(c0, c0 + nsc)
                nc.vector.tensor_mul(mcos, raw, ropc_sb[:, cseg])
                for hb in range(2):
                    E = slice(hb * 64, hb * 64 + 32)
                    O = slice(hb * 64 + 32, hb * 64 + 64)
                    nc.vector.tensor_mul(          # O*sin placed at E rows
                        msw[E], raw[O], rops_sb[O, cseg])
                    nc.vector.tensor_mul(          # E*sin placed at O rows
                        msw[O], raw[E], rops_sb[E, cseg])
                for hb in range(2):
                    E = slice(hb * 64, hb * 64 + 32)
                    O = slice(hb * 64 + 32, hb * 64 + 64)
                    nc.vector.tensor_sub(dst[E], mcos[E], msw[E])
                    nc.vector.tensor_add(dst[O], msw[O], mcos[O])

            def proj_rope_et(w_sb, dst, s0, et):
                # dst[:, et, :] = rope((W^T)^T @ xT[:, s0:SK])
                nsc = SK - s0
                raw = rawp.tile([P, nsc], bf16, name="raw", tag="raw")
                for si, sp in enumerate(range(s0, SK, 512)):
                    ps = projp.tile([P, 512], f32, name="ps", tag="ps")
                    for d in range(8):
                        nc.tensor.matmul(
                            ps,
                            lhsT=w_sb[:, d, et * P:(et + 1) * P],
                            rhs=xt_sb[:, d, sp:sp + 512],
                            start=(d == 0), stop=(d == 7))
                    nc.scalar.copy(out=raw[:, si * 512:(si + 1) * 512],
                                   in_=ps)
                rope(raw, dst[:, et, :], nsc, s0)

            def attention_head(h):
                et, hr = h // 2, (h % 2) * 64
                pv_t = [None, None]
                for kt in range(NKT):
                    lo, hi = max(kt - 4, 0), min(kt, 7)
                    nqb = hi - lo + 1
                    n0 = min(nqb, 4) * P
                    kh = kt_sb[hr:hr + 64, et, kt * P:(kt + 1) * P]
                    # scores S^T[k, q] for q blocks lo..hi in one 2-bank psum
                    st_ps = stp.tile([P, 640], f32, name="st_ps")
                    mms = [(kh, qt_sb[hr:hr + 64, et, lo * P:lo * P + n0],
                            slice(0, n0), True)]
                    if nqb == 5:
                        mms.append((kh,
                                    qt_sb[hr:hr + 64, et,
                                          (lo + 4) * P:(lo + 5) * P],
                                    slice(512, 640), True))
                    if kt >= 4:                       # diag mask @ col 0
                        mms.append((udia_sb, negi_sb, slice(0, P), False))
                    if kt <= 7:                       # oldest mask @ col kt-lo
                        c = (kt - lo) * P
                        mms.append((uold_sb, negi_sb, slice(c, c + P), False))
                    for lhsT, rhs, csl, is_start in mms:
                        nc.tensor.matmul(
                            st_ps[:, csl], lhsT=lhsT, rhs=rhs,
                            start=is_start, stop=not is_start,
                            skip_group_check=True)
                    p_t = ptp.tile([P, 640], bf16, name="p_t")
                    nc.scalar.activation(
                        out=p_t[:, 0:nqb * P], in_=st_ps[:, 0:nqb * P],
                        func=mybir.ActivationFunctionType.Exp, scale=0.125)
                    # PV: one matmul per touched pv bank, batched over qbs.
                    # start=True clears the WHOLE psum bank, so only the very
                    # first matmul into each bank sets it; later groups
                    # overwrite via has_written=0.
                    for g in (0, 1):
                        c0, c1 = max(lo, 4 * g), min(hi, 4 * g + 3)
                        if c0 > c1:
                            continue
                        if pv_t[g] is None:
                            pv_t[g] = pvp.tile([P, 512], f32, name="pvt",
                                               tag="pvt")
                        nc.tensor.matmul(
                            pv_t[g][0:VA, (c0 % 4) * P:(c1 % 4 + 1) * P],
                            lhsT=v_sb[:, kt, h, :],
                            rhs=p_t[:, (c0 - lo) * P:(c1 - lo + 1) * P],
                            start=(kt == 4 * g), stop=(kt == 4 * g + 7),
                            skip_group_check=True)
                    for g in (0, 1):
                        if kt == 4 * g + 7:
                            # retire: normalize by the ones-row denominator;
                            # reciprocal runs lane-parallel via a DMA reshape
                            pv = pv_t[g]
                            rc = epsp.tile([1, 512], f32, name="rc", tag="rc")
                            bc = epsp.tile([64, 512], f32, name="bc",
                                           tag="bc")
                            rcs = epsp.tile([128, 4], f32, name="rcs",
                                            tag="rcs")
                            rcr = epsp.tile([128, 4], f32, name="rcr",
                                            tag="rcr")
                            rrow = epsp.tile([1, 512], f32, name="rrow",
                                             tag="rrow")
                            nc.scalar.copy(out=rrow, in_=pv[64:65, :])
                            nc.sync.dma_start(out=rcs, in_=rrow)
                            nc.vector.reciprocal(rcr, rcs)
                            nc.sync.dma_start(out=rc, in_=rcr)
                            nc.gpsimd.partition_broadcast(bc, rc)
                            nc.vector.tensor_mul(
                                ot_sb[hr:hr + 64, et, g * 512:(g + 1) * 512],
                                pv[0:64, :], bc)

            # ---- projections for the first two etile pairs ----
            proj_rope_et(wk_sb, kt_sb, 0, 0)
            proj_rope_et(wq_sb, qt_sb, W, 0)
            proj_rope_et(wk_sb, kt_sb, 0, 1)
            proj_rope_et(wq_sb, qt_sb, W, 1)

            # ---- V projection (emitted after 2 etiles of K/Q proj so
            # the scalar/vector engines have rope+exp work during it) ----
            with tc.tile_pool(name="wvp", bufs=1) as wvp:
                wv_sb = wvp.tile([P, 8, DIM], bf16)
                nc.sync.dma_start(
                    out=wv_sb, in_=wv.ap().rearrange("(d p) e -> p d e", p=P))
                for st_i in range(NKT):
                    for eh in range(2):
                        ps = projp.tile([P, 512], f32, name="psv", tag="ps")
                        for d in range(8):
                            nc.tensor.matmul(
                                ps,
                                lhsT=xt_sb[:, d, st_i * P:(st_i + 1) * P],
                                rhs=wv_sb[:, d, eh * 512:(eh + 1) * 512],
                                start=(d == 0), stop=(d == 7))
                        # scatter heads into V_aug slots [st, h, 0:64]
                        nc.scalar.copy(
                            out=v_sb[:, st_i, eh * 8:(eh + 1) * 8, 0:64],
                            in_=ps[:, :].rearrange("p (h e) -> p h e", h=8))

            for et in range(8):
                attention_head(2 * et)
                attention_head(2 * et + 1)
                if et + 2 < 8:
                    proj_rope_et(wk_sb, kt_sb, 0, et + 2)
                    proj_rope_et(wq_sb, qt_sb, W, et + 2)

            if dbg_outs is not None:
                nc.sync.dma_start(out=dbg_outs["d_qt"][:, :],
                                  in_=qt_sb[:, :, :])
                nc.sync.dma_start(out=dbg_outs["d_kt"][:, :],
                                  in_=kt_sb[:, :, :])
                nc.sync.dma_start(out=dbg_outs["d_v"][:, :],
                                  in_=v_sb[:, :, :, :])
                nc.sync.dma_start(out=dbg_outs["d_ot"][:, :],
                                  in_=ot_sb[:, :, :])

            # ---- output projection ----
            for qt_i in range(NQB):
                for nh in range(2):
                    ps = pvp.tile([P, 512], f32, name="pso", tag="pvt")
                    for p in range(8):
                        nc.tensor.matmul(
                            ps,
                            lhsT=ot_sb[:, p, qt_i * P:(qt_i + 1) * P],
                            rhs=wo_sb[:, p, nh * 512:(nh + 1) * 512],
                            start=(p == 0), stop=(p == 7))
                    o_sb = osbp.tile([P, 512], f32, name="o_sb")
                    nc.vector.tensor_copy(o_sb, ps)
                    nc.sync.dma_start(
                        out=out[qt_i * P:(qt_i + 1) * P,
                                nh * 512:(nh + 1) * 512],
                        in_=o_sb)


def _prep_inputs(x, Wq, Wk, Wv, Wo):
    """Host-side shard/layout prep -> list of 8 per-core input dicts."""
    x2 = np.ascontiguousarray(x.reshape(S, DIM).astype(np.float32))
    sigma = np.zeros(DIM, dtype=np.int64)
    for h in range(H):
        j = np.arange(32)
        sigma[h * 64 + j] = h * 64 + 2 * j
        sigma[h * 64 + 32 + j] = h * 64 + 2 * j + 1
    wq_h = np.ascontiguousarray(Wq.T[:, sigma]).astype(BF)
    wk_h = np.ascontiguousarray(Wk.T[:, sigma]).astype(BF)
    wv_h = np.ascontiguousarray(Wv.T).astype(BF)
    wo_h = np.ascontiguousarray(Wo.T).astype(BF)

    jj = np.arange(P)
    uold_h = (jj[None, :] <= jj[:, None]).astype(np.float32).astype(BF)
    udia_h = (jj[None, :] >= jj[:, None] + 1).astype(np.float32).astype(BF)
    negi_h = (-1e6 * np.eye(P, dtype=np.float32)).astype(BF)

    inv_freq = 1.0 / (10000.0 ** (np.arange(0, D, 2, dtype=np.float32) / D))
    xT = x2.T  # [DIM, S]

    in_maps = []
    for core in range(NCORES):
        lo = core * SL - W
        xsh = np.zeros((DIM, SK), dtype=np.float32)
        if lo < 0:
            xsh[:, W:] = xT[:, :SL]
        else:
            xsh[:, :] = xT[:, lo:lo + SK]
        pos = np.arange(lo, lo + SK, dtype=np.float32)
        ang = pos[None, :] * inv_freq[:, None]          # [32, SK]
        in_maps.append({
            "xt": xsh.astype(BF),
            "wq": wq_h, "wk": wk_h, "wv": wv_h, "wo": wo_h,
            "ropc": np.ascontiguousarray(
                np.tile(np.cos(ang), (4, 1))).astype(BF),
            "rops": np.ascontiguousarray(
                np.tile(np.sin(ang), (4, 1))).astype(BF),
            "vone": (pos >= 0).astype(np.float32).astype(BF)[:, None],
            "uold": uold_h, "udia": udia_h, "negi": negi_h,
        })
    return in_maps


def kernel(x, Wq, Wk, Wv, Wo, window_size, _trace=False, _trace_kwargs=None):
    assert int(window_size) == W
    if "nc" not in _compiled:
        _compiled["nc"] = _build()
    nc = _compiled["nc"]
    in_maps = _prep_inputs(np.asarray(x), np.asarray(Wq), np.asarray(Wk),
                           np.asarray(Wv), np.asarray(Wo))
    res = run_bass_kernel_spmd(nc, in_maps, core_ids=list(range(NCORES)),
                               trace=_trace, **(_trace_kwargs or {}))
    outp = np.concatenate([res.results[c]["out"] for c in range(NCORES)],
                          axis=0)
    _compiled["last_result"] = res
    return outp.reshape(1, S, DIM).astype(np.float32)


if __name__ == "__main__":
    np.random.seed(0)
    x = np.random.randn(1, S, DIM).astype(np.float32)
    sd = 1.0 / np.sqrt(DIM)
    ws = [np.random.randn(DIM, DIM).astype(np.float32) * sd for _ in range(4)]
    y = kernel(x, *ws, window_size=W)
    print("kernel output", y.shape, y.dtype, np.abs(y).max())



# revision 15
# speedup vs baseline: 1.0592x; 1.0592x over previous
"""Self-contained Trainium2 Bass kernel for sliding-window attention.

Problem (hardcoded): B=1, S=8192, dim=1024, H=16 heads, D=64 head dim,
window=512, fp32 I/O.  y = (softmax(mask(rope(xWq^T) rope(xWk^T)^T / 8)) xWv^T) Wo^T

Strategy: sequence-parallel over 8 NeuronCores. Each core owns 1024 query
rows and additionally recomputes K/V for the 512-row halo to its left
(core 0's halo is zero-padded and neutralized via a per-core "vone"
column so no collective is needed).  All matmuls run in bf16 (fp32 PSUM
accumulation); sliding-window causality is applied by extra rank-128
mask matmuls accumulated into the score PSUM before the exp.

Layouts (per core):
  xT    [1024(d), 1536(s)]  x^T shard incl. halo (bf16)
  wq/wk [1024(d), 1024(e')] Wq^T / Wk^T with a per-head even/odd column
                            permutation (rope pair de-interleave)
  Q^T/K^T are produced in [e', s] layout (weight-stationary matmuls) so
  attention needs no transposes: scores are computed transposed,
  S^T[k, q], the softmax denominator comes free from a ones-column
  appended to V, and PV directly yields o^T[e, q] -- the lhsT of the
  output projection.

Perf structure: rope uses sign-baked sin tables (6 DVE ops per etile);
the PE instruction stream is software-pipelined (PV lags scores by 2
slots) with the next etile's projection matmuls woven in as filler;
HBM parameters are laid out chunk-contiguous and DMA'd in consumption
order so the PE starts within a few microseconds.
"""
import sys

sys.path.insert(0, "/opt/trn_rl_repo")

import numpy as np
import ml_dtypes

import concourse.bass as bass
import concourse.mybir as mybir
from concourse import bacc
from concourse.tile import TileContext
from concourse.bass_utils import run_bass_kernel_spmd

BF = ml_dtypes.bfloat16
NCORES = 8
S, DIM, H, D, W = 8192, 1024, 16, 64, 512
SL = S // NCORES          # 1024 own rows / core
SK = SL + W               # 1536 rows incl. left halo
P = 128
NKT = SK // P             # 12 kv tiles
NQB = SL // P             # 8 query tiles
dt = mybir.dt

VA = 80   # V_aug columns: 64 V + 1 ones + pad (32B-aligned stride)

# table column offsets (bf16 columns in the packed tbl parameter)
T_COS, T_SIN = 0, SK
T_UOLD, T_UDIA, T_NEGI = 2 * SK, 2 * SK + P, 2 * SK + 2 * P
T_VONE = 2 * SK + 3 * P
T_PAD = 3584

_compiled = {}


def _build(dbg=False):
    nc = bacc.Bacc("TRN2", target_bir_lowering=False, debug=False,
                   num_devices=NCORES)
    def param(name, shape, dtype=dt.bfloat16, out=False):
        return nc.declare_dram_parameter(name, shape, dtype, isOutput=out)

    xt3 = param("xt3", [3, P, 8, 512])
    wk8 = param("wk8", [8, P, 8, P])
    wq8 = param("wq8", [8, P, 8, P])
    wv2 = param("wv2", [2, P, 8, 512])
    wo2 = param("wo2", [2, P, 8, 512])
    tbl = param("tbl", [P, T_PAD])
    out = param("out", [SL, DIM], dt.float32, out=True)
    dbg_outs = None
    if dbg:
        dbg_outs = {
            "d_qt": param("d_qt", [P, 8 * SL], out=True),
            "d_kt": param("d_kt", [P, 8 * SK], out=True),
            "d_v": param("d_v", [P, NKT * H * VA], out=True),
            "d_ot": param("d_ot", [P, 8 * SL], out=True),
        }

    with TileContext(nc) as tc:
        _body(nc, tc, xt3, wk8, wq8, wv2, wo2, tbl, out, dbg_outs)
    nc.compile()
    return nc


def _brd2(ap_slice, n):
    """Insert a stride-0 middle free dim of size n into a [p, c] AP."""
    return bass.AP(tensor=ap_slice.tensor, offset=ap_slice.offset,
                   ap=[ap_slice.ap[0], [0, n], ap_slice.ap[1]])


def _body(nc, tc, xt3, wk8, wq8, wv2, wo2, tbl, out, dbg_outs=None):
    f32, bf16 = dt.float32, dt.bfloat16

    with tc.tile_pool(name="persist", bufs=1) as per:
        v_sb = per.tile([P, NKT, H, VA], bf16)    # V_aug: [V(64)|ones|pad]
        qt_sb = per.tile([P, 8, SL], bf16)        # Q^T (rope'd, sigma layout)
        kt_sb = per.tile([P, 8, SK], bf16)        # K^T
        ot_sb = per.tile([P, 8, SL], bf16)        # o^T (normalized)
        tbl_sb = per.tile([P, T_PAD], bf16)
        uold_sb = tbl_sb[:, T_UOLD:T_UOLD + P]
        udia_sb = tbl_sb[:, T_UDIA:T_UDIA + P]
        negi_sb = tbl_sb[:, T_NEGI:T_NEGI + P]
        vone_sb = tbl_sb[:, T_VONE:T_VONE + NKT]

        with tc.tile_pool(name="xtp", bufs=1) as xtp, \
             tc.tile_pool(name="wqk", bufs=1) as wqkp, \
             tc.tile_pool(name="proj", bufs=1, space="PSUM") as projp, \
             tc.tile_pool(name="raw", bufs=2) as rawp, \
             tc.tile_pool(name="mro", bufs=1) as mrp, \
             tc.tile_pool(name="pt", bufs=3) as ptp, \
             tc.tile_pool(name="st", bufs=2, space="PSUM") as stp, \
             tc.tile_pool(name="pv", bufs=3, space="PSUM") as pvp, \
             tc.tile_pool(name="osb", bufs=2) as osbp, \
             tc.tile_pool(name="eps", bufs=1) as epsp:
            xt_sb = xtp.tile([P, 8, SK], bf16)
            wk_sb = wqkp.tile([P, 8, DIM], bf16)
            wq_sb = wqkp.tile([P, 8, DIM], bf16)
            wv_sb = wqkp.tile([P, 8, DIM], bf16)
            wo_sb = wqkp.tile([P, 8, DIM], bf16)

            # ---- input DMAs, split for queue parallelism, in
            #      consumption order across the three DGE rings ----
            xr = [xt3.ap()[sc] for sc in range(3)]
            nc.sync.dma_start(out=xt_sb[:, 0:4, 0:512], in_=xr[0][:, 0:4])
            nc.sync.dma_start(out=xt_sb[:, 4:8, 0:512], in_=xr[0][:, 4:8])
            nc.scalar.dma_start(out=wk_sb[:, :, 0:P], in_=wk8.ap()[0])
            nc.gpsimd.dma_start(out=tbl_sb, in_=tbl[:, :])
            nc.sync.dma_start(out=xt_sb[:, 0:4, 512:1024], in_=xr[1][:, 0:4])
            nc.sync.dma_start(out=xt_sb[:, 4:8, 512:1024], in_=xr[1][:, 4:8])
            nc.scalar.dma_start(out=wq_sb[:, :, 0:P], in_=wq8.ap()[0])
            nc.sync.dma_start(out=xt_sb[:, 0:4, 1024:1536], in_=xr[2][:, 0:4])
            nc.sync.dma_start(out=xt_sb[:, 4:8, 1024:1536], in_=xr[2][:, 4:8])
            nc.scalar.dma_start(out=wk_sb[:, :, P:2 * P], in_=wk8.ap()[1])
            nc.scalar.dma_start(out=wq_sb[:, :, P:2 * P], in_=wq8.ap()[1])
            for eh in range(2):
                for dh in range(2):
                    nc.gpsimd.dma_start(
                        out=wv_sb[:, 4 * dh:4 * dh + 4,
                                  eh * 512:(eh + 1) * 512],
                        in_=wv2.ap()[eh][:, 4 * dh:4 * dh + 4])
            for et in range(2, 8):
                eng = nc.sync if et % 2 == 0 else nc.scalar
                eng.dma_start(out=wk_sb[:, :, et * P:(et + 1) * P],
                              in_=wk8.ap()[et])
                eng.dma_start(out=wq_sb[:, :, et * P:(et + 1) * P],
                              in_=wq8.ap()[et])
            for nh in range(2):
                nc.sync.dma_start(out=wo_sb[:, :, nh * 512:(nh + 1) * 512],
                                  in_=wo2.ap()[nh])

            # ones column of V_aug (per-core halo-validity mask)
            for kt_i in range(NKT):
                nc.vector.tensor_copy(
                    out=v_sb[:, kt_i, :, 64],
                    in_=_brd2(vone_sb[:, kt_i:kt_i + 1], H))

            # ---- projection machinery (weavable units) ----
            def proj_units(w_sb, dst, s0, et, pool=None, tag="ps",
                           raw_eng=None):
                """Units for one etile of K or Q projection + rope. Each
                matmul unit emits 4 PE matmuls; chunk-closing units add the
                psum->sbuf copy; a final unit emits 6 DVE rope ops over the
                full etile."""
                pool = pool or projp
                nsc = SK - s0
                raw = rawp.tile([P, SK], bf16, name="raw", tag="raw")
                units = []
                for sp in range(s0, SK, 512):
                    ps = pool.tile([P, 512], f32, name="ps", tag=tag)

                    def mk_mm(d0, ps=ps, sp=sp):
                        def emit():
                            for d in range(d0, d0 + 4):
                                nc.tensor.matmul(
                                    ps,
                                    lhsT=w_sb[:, d, et * P:(et + 1) * P],
                                    rhs=xt_sb[:, d, sp:sp + 512],
                                    start=(d == 0), stop=(d == 7),
                                    skip_group_check=True)
                        return emit

                    def mk_copy(ps=ps, sp=sp):
                        def emit():
                            c = sp - s0
                            eng = raw_eng or nc.scalar
                            if eng is nc.scalar:
                                eng.copy(out=raw[:, c:c + 512], in_=ps)
                            else:
                                eng.tensor_copy(out=raw[:, c:c + 512],
                                                in_=ps)
                        return emit

                    units.append((mk_mm(0), None))
                    units.append((mk_mm(4), mk_copy()))

                def mk_rope():
                    def emit():
                        cseg = slice(T_COS + s0, T_COS + SK)
                        sseg = slice(T_SIN + s0, T_SIN + SK)
                        msw = mrp.tile([P, SK], bf16, name="msw", tag="msw")
                        for hb in range(2):
                            E = slice(hb * 64, hb * 64 + 32)
                            O = slice(hb * 64 + 32, hb * 64 + 64)
                            nc.vector.tensor_mul(
                                msw[E, 0:nsc], raw[O, 0:nsc],
                                tbl_sb[O, sseg])
                            nc.vector.tensor_mul(
                                msw[O, 0:nsc], raw[E, 0:nsc],
                                tbl_sb[E, sseg])
                        rw = raw[:, 0:nsc]
                        nc.vector.tensor_mul(rw, rw, tbl_sb[:, cseg])
                        nc.vector.tensor_add(
                            dst[:, et, 0:nsc], rw, msw[:, 0:nsc])
                    return emit

                units.append((mk_rope(), None))
                return units

            def run_units(units):
                for mm, post in units:
                    mm()
                    if post is not None:
                        post()

            # ---- upfront: K/Q projections for etiles 0,1 (borrow the
            #      3-deep pv psum slots, idle until attention) ----
            run_units(proj_units(wk_sb, kt_sb, 0, 0, pool=pvp, tag="pvt"))
            run_units(proj_units(wq_sb, qt_sb, W, 0, pool=pvp, tag="pvt"))
            run_units(proj_units(wk_sb, kt_sb, 0, 1, pool=pvp, tag="pvt"))
            run_units(proj_units(wq_sb, qt_sb, W, 1, pool=pvp, tag="pvt"))

            # ---- V projection (scatter copies on ACT) ----
            for st_i in range(NKT):
                for eh in range(2):
                    ps = pvp.tile([P, 512], f32, name="psv", tag="pvt")
                    for d in range(8):
                        nc.tensor.matmul(
                            ps,
                            lhsT=xt_sb[:, d, st_i * P:(st_i + 1) * P],
                            rhs=wv_sb[:, d, eh * 512:(eh + 1) * 512],
                            start=(d == 0), stop=(d == 7),
                            skip_group_check=True)
                    nc.scalar.copy(
                        out=v_sb[:, st_i, eh * 8:(eh + 1) * 8, 0:64],
                        in_=ps[:, :].rearrange("p (h e) -> p h e", h=8))

            # ---- attention: 2 heads per iter, PV lagged 2 slots, proj
            #      units for etile et+2 woven in as PE filler ----
            pv_t = {}          # head -> [g0 tile, g1 tile]
            p_tiles = {}       # slot idx -> (h, kt, p_t tile, lo, hi)
            slot_no = 0

            def emit_scores(h, kt):
                et, hr = h // 2, (h % 2) * 64
                lo, hi = max(kt - 4, 0), min(kt, 7)
                nqb = hi - lo + 1
                n0 = min(nqb, 4) * P
                kh = kt_sb[hr:hr + 64, et, kt * P:(kt + 1) * P]
                st_ps = stp.tile([P, 640], f32, name="st_ps")
                mms = [(kh, qt_sb[hr:hr + 64, et, lo * P:lo * P + n0],
                        slice(0, n0), True)]
                if nqb == 5:
                    mms.append((kh,
                                qt_sb[hr:hr + 64, et,
                                      (lo + 4) * P:(lo + 5) * P],
                                slice(512, 640), True))
                if kt >= 4:                       # diag mask @ col 0
                    mms.append((udia_sb, negi_sb, slice(0, P), False))
                if kt <= 7:                       # oldest mask @ col kt-lo
                    c = (kt - lo) * P
                    mms.append((uold_sb, negi_sb, slice(c, c + P), False))
                for lhsT, rhs, csl, is_start in mms:
                    nc.tensor.matmul(
                        st_ps[:, csl], lhsT=lhsT, rhs=rhs,
                        start=is_start, stop=not is_start,
                        skip_group_check=True)
                p_t = ptp.tile([P, 640], bf16, name="p_t")
                nc.scalar.activation(
                    out=p_t[:, 0:nqb * P], in_=st_ps[:, 0:nqb * P],
                    func=mybir.ActivationFunctionType.Exp, scale=0.125)
                return p_t, lo, hi

            def emit_pv(h, kt, p_t, lo, hi):
                et, hr = h // 2, (h % 2) * 64
                if h not in pv_t:
                    pv_t[h] = [None, None]
                for g in (0, 1):
                    c0, c1 = max(lo, 4 * g), min(hi, 4 * g + 3)
                    if c0 > c1:
                        continue
                    if pv_t[h][g] is None:
                        pv_t[h][g] = pvp.tile([P, 512], f32, name="pvt",
                                              tag="pvt")
                    nc.tensor.matmul(
                        pv_t[h][g][0:VA, (c0 % 4) * P:(c1 % 4 + 1) * P],
                        lhsT=v_sb[:, kt, h, :],
                        rhs=p_t[:, (c0 - lo) * P:(c1 - lo + 1) * P],
                        start=(kt == 4 * g), stop=(kt == 4 * g + 7),
                        skip_group_check=True)
                for g in (0, 1):
                    if kt == 4 * g + 7:
                        # retire: normalize by the ones-row denominator;
                        # reciprocal runs lane-parallel via a DMA reshape
                        pv = pv_t[h][g]
                        rc = epsp.tile([1, 512], f32, name="rc", tag="rc")
                        bc = epsp.tile([64, 512], f32, name="bc", tag="bc")
                        rcs = epsp.tile([128, 4], f32, name="rcs", tag="rcs")
                        rcr = epsp.tile([128, 4], f32, name="rcr", tag="rcr")
                        rrow = epsp.tile([1, 512], f32, name="rrow",
                                         tag="rrow")
                        nc.vector.tensor_copy(out=rrow, in_=pv[64:65, :])
                        nc.sync.dma_start(out=rcs, in_=rrow)
                        nc.vector.reciprocal(rcr, rcs)
                        nc.sync.dma_start(out=rc, in_=rcr)
                        nc.gpsimd.partition_broadcast(bc, rc)
                        nc.vector.tensor_mul(
                            ot_sb[hr:hr + 64, et, g * 512:(g + 1) * 512],
                            pv[0:64, :], bc)
                        pv_t[h][g] = None

            LAG = 2
            for et in range(8):
                fillers = []
                if et + 2 < 8:
                    fillers += proj_units(wk_sb, kt_sb, 0, et + 2)
                    fillers += proj_units(wq_sb, qt_sb, W, et + 2,
                                          raw_eng=nc.vector)
                slots = [(h, kt) for h in (2 * et, 2 * et + 1)
                         for kt in range(NKT)]
                nf = len(fillers)
                done_f = 0
                for i, (h, kt) in enumerate(slots):
                    p_t, lo, hi = emit_scores(h, kt)
                    p_tiles[slot_no] = (h, kt, p_t, lo, hi)
                    want = (i + 1) * nf // len(slots)
                    while done_f < want:
                        mm, post = fillers[done_f]
                        mm()
                        if post is not None:
                            post()
                        done_f += 1
                    if slot_no - LAG in p_tiles:
                        ph, pkt, pp, plo, phi = p_tiles.pop(slot_no - LAG)
                        emit_pv(ph, pkt, pp, plo, phi)
                    slot_no += 1
                if et == 7:   # drain the lagged PV slots
                    for s in sorted(p_tiles):
                        ph, pkt, pp, plo, phi = p_tiles.pop(s)
                        emit_pv(ph, pkt, pp, plo, phi)

            if dbg_outs is not None:
                nc.sync.dma_start(out=dbg_outs["d_qt"][:, :], in_=qt_sb[:, :, :])
                nc.sync.dma_start(out=dbg_outs["d_kt"][:, :], in_=kt_sb[:, :, :])
                nc.sync.dma_start(out=dbg_outs["d_v"][:, :],
                                  in_=v_sb[:, :, :, :])
                nc.sync.dma_start(out=dbg_outs["d_ot"][:, :], in_=ot_sb[:, :, :])

            # ---- output projection ----
            for qt_i in range(NQB):
                for nh in range(2):
                    ps = pvp.tile([P, 512], f32, name="pso", tag="pvt")
                    for p in range(8):
                        nc.tensor.matmul(
                            ps,
                            lhsT=ot_sb[:, p, qt_i * P:(qt_i + 1) * P],
                            rhs=wo_sb[:, p, nh * 512:(nh + 1) * 512],
                            start=(p == 0), stop=(p == 7),
                            skip_group_check=True)
                    o_sb = osbp.tile([P, 512], f32, name="o_sb")
                    nc.vector.tensor_copy(o_sb, ps)
                    nc.sync.dma_start(
                        out=out[qt_i * P:(qt_i + 1) * P,
                                nh * 512:(nh + 1) * 512],
                        in_=o_sb)


def _prep_inputs(x, Wq, Wk, Wv, Wo):
    """Host-side shard/layout prep -> list of 8 per-core input dicts."""
    x2 = np.ascontiguousarray(x.reshape(S, DIM).astype(np.float32))
    sigma = np.zeros(DIM, dtype=np.int64)
    for h in range(H):
        j = np.arange(32)
        sigma[h * 64 + j] = h * 64 + 2 * j
        sigma[h * 64 + 32 + j] = h * 64 + 2 * j + 1
    wq_h = np.ascontiguousarray(Wq.T[:, sigma]).astype(np.float32)
    wk_h = np.ascontiguousarray(Wk.T[:, sigma]).astype(np.float32)
    wv_h = np.ascontiguousarray(Wv.T).astype(np.float32)
    wo_h = np.ascontiguousarray(Wo.T).astype(np.float32)

    def chunk4(w, ncol):
        nc_ = DIM // ncol
        return np.ascontiguousarray(
            w.reshape(8, P, nc_, ncol).transpose(2, 1, 0, 3)).astype(BF)

    wk8_h = chunk4(wk_h, P)
    wq8_h = chunk4(wq_h, P)
    wv2_h = chunk4(wv_h, 512)
    wo2_h = chunk4(wo_h, 512)

    jj = np.arange(P)
    uold_h = (jj[None, :] <= jj[:, None]).astype(np.float32)
    udia_h = (jj[None, :] >= jj[:, None] + 1).astype(np.float32)
    negi_h = -1e6 * np.eye(P, dtype=np.float32)

    inv_freq = 1.0 / (10000.0 ** (np.arange(0, D, 2, dtype=np.float32) / D))
    xT = x2.T  # [DIM, S]
    # sign-baked sin table: +sin on even-half rows (E), -sin on odd-half (O)
    sign = np.where((np.arange(P) % 64) < 32, 1.0, -1.0).astype(np.float32)

    in_maps = []
    for core in range(NCORES):
        lo = core * SL - W
        xsh = np.zeros((DIM, SK), dtype=np.float32)
        if lo < 0:
            xsh[:, W:] = xT[:, :SL]
        else:
            xsh[:, :] = xT[:, lo:lo + SK]
        xt3_h = np.ascontiguousarray(
            xsh.reshape(8, P, 3, 512).transpose(2, 1, 0, 3)).astype(BF)
        pos = np.arange(lo, lo + SK, dtype=np.float32)
        ang = pos[None, :] * inv_freq[:, None]          # [32, SK]
        ropc = np.tile(np.cos(ang), (4, 1))             # [128, SK]
        rops = np.tile(np.sin(ang), (4, 1)) * sign[:, None]
        vone = (pos.reshape(NKT, P).T >= 0).astype(np.float32)
        tbl_h = np.zeros((P, T_PAD), dtype=np.float32)
        tbl_h[:, T_COS:T_COS + SK] = ropc
        tbl_h[:, T_SIN:T_SIN + SK] = rops
        tbl_h[:, T_UOLD:T_UOLD + P] = uold_h
        tbl_h[:, T_UDIA:T_UDIA + P] = udia_h
        tbl_h[:, T_NEGI:T_NEGI + P] = negi_h
        tbl_h[:, T_VONE:T_VONE + NKT] = vone
        in_maps.append({
            "xt3": xt3_h,
            "wk8": wk8_h, "wq8": wq8_h, "wv2": wv2_h, "wo2": wo2_h,
            "tbl": tbl_h.astype(BF),
        })
    return in_maps


def kernel(x, Wq, Wk, Wv, Wo, window_size, _trace=False, _trace_kwargs=None):
    assert int(window_size) == W
    if "nc" not in _compiled:
        _compiled["nc"] = _build()
    nc = _compiled["nc"]
    in_maps = _prep_inputs(np.asarray(x), np.asarray(Wq), np.asarray(Wk),
                           np.asarray(Wv), np.asarray(Wo))
    res = run_bass_kernel_spmd(nc, in_maps, core_ids=list(range(NCORES)),
                               trace=_trace, **(_trace_kwargs or {}))
    outp = np.concatenate([res.results[c]["out"] for c in range(NCORES)],
                          axis=0)
    _compiled["last_result"] = res
    return outp.reshape(1, S, DIM).astype(np.float32)


if __name__ == "__main__":
    np.random.seed(0)
    x = np.random.randn(1, S, DIM).astype(np.float32)
    sd = 1.0 / np.sqrt(DIM)
    ws = [np.random.randn(DIM, DIM).astype(np.float32) * sd for _ in range(4)]
    y = kernel(x, *ws, window_size=W)
    print("kernel output", y.shape, y.dtype, np.abs(y).max())
